# revision 10
# baseline (speedup 1.0000x reference)
"""AJ-RNN (2-layer LSTM with missing-value imputation) Trainium2 Bass kernel.

Strategy: data-parallel over batch across 8 NeuronCores (B=128 -> 16/core).
The recurrence is sequential in T, so everything is laid out TRANSPOSED
(zT layout: gate/hidden rows on partitions, batch on the free axis) so that
elementwise work uses all 128 partitions and the weights act as the matmul
stationary operand (lhsT) with the small batch as the moving free dim.

  z1T = b0-row + R0^T @ h1T + K0^T @ curT      (PSUM [128, 16*16], 16 row-tiles)
  z2T = b1-row + R1^T @ h2T + K1^T @ h1T
  gates: one sigmoid per layer (g-gate columns pre-scaled by 2 on the host:
         tanh(g) = 2*sigmoid(2g) - 1), fp32 gate values, fp32 c state,
         bf16 h for the big matmuls, fp32 h2 for the pred/output path.
  pred(t) = bias-row + W^T @ h2f  -- doubles as the imputation source AND the
         prediction output rows (pred at step t = output row t-1).

Per-step emission order is chosen so the parts depending only on state from
t-1 (R0/R1/bias matmuls, pred) are scheduled early, and only the K0(cur)/K1(h1)
tails sit on the serial chain.

No cross-core communication: each core runs its 16 sequences independently;
the host shards inputs and concatenates outputs.
"""

import numpy as np
import ml_dtypes

import concourse.bass as bass
import concourse.tile as tile
from concourse import bacc, mybir
from concourse.bass_utils import run_bass_kernel_spmd
from concourse.masks import make_identity

BF = ml_dtypes.bfloat16
B, T, D, H = 128, 256, 64, 512
NCORES = 8
BL = B // NCORES          # 16 batch rows per core
G4 = 4 * H                # 2048
NT = G4 // 128            # 16 z row-tiles
KC = H // 128             # 4 contraction chunks


def build(Tsteps=T, bl=BL):
    f32 = mybir.dt.float32
    bf16 = mybir.dt.bfloat16
    AF = mybir.ActivationFunctionType
    nc = bacc.Bacc("TRN2", target_bir_lowering=False, debug=False, num_devices=NCORES)

    # --- DRAM parameters (per-core shards; weights identical on all cores) ---
    d_r0w = nc.dram_tensor("r0w", [128, KC * G4], bf16, kind="ExternalInput")
    d_k1w = nc.dram_tensor("k1w", [128, KC * G4], bf16, kind="ExternalInput")
    d_r1w = nc.dram_tensor("r1w", [128, KC * G4], bf16, kind="ExternalInput")
    d_k0w = nc.dram_tensor("k0w", [D, G4], bf16, kind="ExternalInput")
    d_b0w = nc.dram_tensor("b0w", [1, G4], bf16, kind="ExternalInput")
    d_b1w = nc.dram_tensor("b1w", [1, G4], bf16, kind="ExternalInput")
    d_wc = nc.dram_tensor("wc", [128, KC * D], f32, kind="ExternalInput")
    d_bias = nc.dram_tensor("biasc", [1, D], f32, kind="ExternalInput")
    d_xt = nc.dram_tensor("xt", [D, Tsteps * bl], bf16, kind="ExternalInput")
    d_mt = nc.dram_tensor("mt", [D, Tsteps * bl], mybir.dt.uint8, kind="ExternalInput")
    d_pred = nc.dram_tensor("pred_out", [(Tsteps - 1) * bl, D], f32, kind="ExternalOutput")
    d_lc = nc.dram_tensor("lc_out", [bl, H], f32, kind="ExternalOutput")

    from contextlib import ExitStack
    with tile.TileContext(nc) as tc, ExitStack() as st:
        wpool = st.enter_context(tc.tile_pool(name="weights", bufs=1))
        spool = st.enter_context(tc.tile_pool(name="state", bufs=1))
        gpool = st.enter_context(tc.tile_pool(name="gates", bufs=3))
        tpool = st.enter_context(tc.tile_pool(name="tmp", bufs=4))
        cpool = st.enter_context(tc.tile_pool(name="cur", bufs=3))
        zpool = st.enter_context(tc.tile_pool(name="zpsum", bufs=2, space="PSUM"))
        ppool = st.enter_context(tc.tile_pool(name="ppsum", bufs=2, space="PSUM"))
        epool = st.enter_context(tc.tile_pool(name="epsum", bufs=2, space="PSUM"))

        # --- load weights/input once ---
        r0w = wpool.tile([128, KC * G4], bf16)
        k1w = wpool.tile([128, KC * G4], bf16)
        r1w = wpool.tile([128, KC * G4], bf16)
        k0w = wpool.tile([D, G4], bf16)
        b0w = wpool.tile([1, G4], bf16)
        b1w = wpool.tile([1, G4], bf16)
        wc = wpool.tile([128, KC * D], f32)
        biasc = wpool.tile([1, D], f32)
        xt = wpool.tile([D, Tsteps * bl], bf16)
        mt = wpool.tile([D, Tsteps * bl], mybir.dt.uint8)
        ident = wpool.tile([128, 128], f32)
        ones = wpool.tile([1, bl], bf16)
        onesf = wpool.tile([1, bl], f32)
        nc.sync.dma_start(r0w[:], d_r0w[:])
        nc.sync.dma_start(k1w[:], d_k1w[:])
        nc.sync.dma_start(r1w[:], d_r1w[:])
        nc.sync.dma_start(k0w[:], d_k0w[:])
        nc.sync.dma_start(b0w[:], d_b0w[:])
        nc.sync.dma_start(b1w[:], d_b1w[:])
        nc.sync.dma_start(wc[:], d_wc[:])
        nc.sync.dma_start(biasc[:], d_bias[:])
        nc.sync.dma_start(xt[:], d_xt[:])
        nc.sync.dma_start(mt[:], d_mt[:])
        make_identity(nc, ident[:])
        nc.gpsimd.memset(ones[:], 1.0)
        nc.gpsimd.memset(onesf[:], 1.0)

        # --- state ---
        h1b = spool.tile([128, KC * bl], bf16)    # h1 transposed, bf16
        h2b = spool.tile([128, KC * bl], bf16)
        h2f = spool.tile([128, KC * bl], f32)     # fp32 h2 for pred/output
        c1 = spool.tile([128, KC * bl], f32)
        c2 = spool.tile([128, KC * bl], f32)
        slab = spool.tile([D, (Tsteps - 1) * bl], f32)  # pred outputs, col t-1
        nc.vector.memset(h1b[:], 0.0)
        nc.vector.memset(h2b[:], 0.0)
        nc.vector.memset(h2f[:], 0.0)
        nc.vector.memset(c1[:], 0.0)
        nc.vector.memset(c2[:], 0.0)

        def layer(zp, s, c_st, hb, hf):
            """gates + state update from accumulated zT psum."""
            nc.scalar.activation(s[:], zp[:], AF.Sigmoid)
            gi = s[:, 0:4 * bl]
            gf = s[:, 4 * bl:8 * bl]
            gg = s[:, 8 * bl:12 * bl]
            go = s[:, 12 * bl:16 * bl]
            t0 = tpool.tile([128, KC * bl], f32, tag="t0")
            t1 = tpool.tile([128, KC * bl], f32, tag="t1")
            th = tpool.tile([128, KC * bl], f32, tag="th")
            nc.vector.tensor_mul(t0[:], gi, gg)                     # i*g'
            nc.vector.tensor_mul(t1[:], gf, c_st[:])                # f*c
            nc.vector.scalar_tensor_tensor(                         # 2*t0 + t1
                t1[:], t0[:], 2.0, t1[:], mybir.AluOpType.mult, mybir.AluOpType.add)
            nc.vector.tensor_sub(c_st[:], t1[:], gi)                # c = 2ig' + fc - i
            nc.scalar.activation(th[:], c_st[:], AF.Tanh)
            nc.vector.tensor_mul(hb[:], th[:], go)                  # bf16 h
            if hf is not None:
                nc.vector.tensor_mul(hf[:], th[:], go)              # fp32 h

        for t in range(Tsteps):
            # ---- early work: depends only on state(t-1) ----
            z1p = zpool.tile([128, NT * bl], f32, tag="z1")
            for j in range(NT):
                out = z1p[:, j * bl:(j + 1) * bl]
                nc.tensor.matmul(out, b0w[0:1, j * 128:(j + 1) * 128], ones[:],
                                 start=(j == 0), stop=False)
                for k in range(KC):
                    nc.tensor.matmul(out, r0w[:, (k * G4 + j * 128):(k * G4 + j * 128 + 128)],
                                     h1b[:, k * bl:(k + 1) * bl],
                                     start=False, stop=False)
            z2p = zpool.tile([128, NT * bl], f32, tag="z2")
            for j in range(NT):
                out = z2p[:, j * bl:(j + 1) * bl]
                nc.tensor.matmul(out, b1w[0:1, j * 128:(j + 1) * 128], ones[:],
                                 start=(j == 0), stop=False)
                for k in range(KC):
                    nc.tensor.matmul(out, r1w[:, (k * G4 + j * 128):(k * G4 + j * 128 + 128)],
                                     h2b[:, k * bl:(k + 1) * bl],
                                     start=False, stop=False)

            # ---- prediction from h2f(t-1) (includes output bias) ----
            cur = cpool.tile([D, bl], bf16, tag="cur")
            if t > 0:
                pp = ppool.tile([D, bl], f32, tag="pp")
                nc.tensor.matmul(pp[:], biasc[:], onesf[:], start=True, stop=False)
                for k in range(KC):
                    nc.tensor.matmul(pp[:], wc[:, k * D:(k + 1) * D],
                                     h2f[:, k * bl:(k + 1) * bl],
                                     start=False, stop=(k == KC - 1))
                # cur = where(mask, pred, x); pred read straight from PSUM
                nc.vector.tensor_copy(cur[:], xt[:, t * bl:(t + 1) * bl])
                nc.vector.copy_predicated(cur[:], mt[:, t * bl:(t + 1) * bl], pp[:])
            else:
                nc.vector.tensor_copy(cur[:], xt[:, 0:bl])

            # ---- layer 1 tail: K0 @ cur, then gates ----
            for j in range(NT):
                nc.tensor.matmul(z1p[:, j * bl:(j + 1) * bl],
                                 k0w[:, j * 128:(j + 1) * 128], cur[:],
                                 start=False, stop=(j == NT - 1))
            s1 = gpool.tile([128, NT * bl], f32, tag="s1")
            layer(z1p, s1, c1, h1b, None)
            if t > 0:
                # prediction output row t-1 (off the critical chain)
                nc.scalar.copy(slab[:, (t - 1) * bl:t * bl], pp[:])

            # ---- layer 2 tail: K1 @ h1(t), then gates ----
            for j in range(NT):
                out = z2p[:, j * bl:(j + 1) * bl]
                for k in range(KC):
                    nc.tensor.matmul(out, k1w[:, (k * G4 + j * 128):(k * G4 + j * 128 + 128)],
                                     h1b[:, k * bl:(k + 1) * bl],
                                     start=False, stop=(j == NT - 1 and k == KC - 1))
            s2 = gpool.tile([128, NT * bl], f32, tag="s2")
            layer(z2p, s2, c2, h2b, h2f)

        # ---- outputs ----
        # prediction slab [D, (T-1)*bl] -> [(T-1)*bl, D]: for each batch row b,
        # transpose [D, 128-t-block] (strided cols t*bl+b) -> [128 t, D] and DMA
        # to the contiguous DRAM rows b*(T-1)+t0 .. +nt.
        dmaq = [nc.sync, nc.scalar, nc.gpsimd]
        Tm1 = Tsteps - 1
        nchunk = (Tm1 + 127) // 128
        qi = 0
        for b in range(bl):
            for ci in range(nchunk):
                t0c = ci * 128
                nt_c = min(128, Tm1 - t0c)
                src = slab[:, (t0c * bl + b)::bl][:, 0:nt_c]
                tp = epool.tile([128, D], f32, tag="tp")
                nc.tensor.transpose(tp[0:nt_c, :], src, ident[0:D, 0:D])
                ob = tpool.tile([128, D], f32, tag="ob")
                nc.scalar.copy(ob[0:nt_c, :], tp[0:nt_c, :])
                dmaq[qi % len(dmaq)].dma_start(
                    d_pred[b * Tm1 + t0c: b * Tm1 + t0c + nt_c, :], ob[0:nt_c, :])
                qi += 1

        # last_cell = h2(T-1): h2f tiles [128, bl] -> [bl, 128]
        lcb = tpool.tile([bl, H], f32, tag="lcb")
        for k in range(KC):
            lp = epool.tile([bl, 128], f32, tag="tp")
            nc.tensor.transpose(lp[:], h2f[:, k * bl:(k + 1) * bl], ident[:])
            nc.scalar.copy(lcb[:, k * 128:(k + 1) * 128], lp[:])
        nc.sync.dma_start(d_lc[:], lcb[:])

    nc.compile()
    return nc


def prep_shared(k0, r0, b0, k1, r1, b1, W, bias):
    """Host-side weight preprocessing (shared across cores)."""
    def gscale(m):
        m = np.array(m, dtype=np.float32, copy=True)
        m[..., 2 * H:3 * H] *= 2.0
        return m

    def chunk(m):  # [K, G4] -> [128, (K/128)*G4], chunk k at cols [k*G4, (k+1)*G4)
        K = m.shape[0]
        return np.ascontiguousarray(
            m.reshape(K // 128, 128, G4).transpose(1, 0, 2).reshape(128, -1))

    return dict(
        r0w=chunk(gscale(r0)).astype(BF),
        k1w=chunk(gscale(k1)).astype(BF),
        r1w=chunk(gscale(r1)).astype(BF),
        k0w=gscale(k0).astype(BF),
        b0w=gscale(b0)[None, :].astype(BF),
        b1w=gscale(b1)[None, :].astype(BF),
        wc=np.ascontiguousarray(
            np.asarray(W, np.float32).reshape(KC, 128, D).transpose(1, 0, 2).reshape(128, KC * D)),
        biasc=np.asarray(bias, np.float32).reshape(1, D),
    )


def prep_core(x_core, Tsteps=T, bl=BL):
    """Per-core x shard -> transposed bf16 x and mask tensors [D, T*bl]."""
    x_core = np.asarray(x_core, np.float32)          # [bl, T, D]
    m = (x_core == 128.0)
    m[:, 0, :] = False                                # t=0: no imputation
    xtr = x_core.transpose(2, 1, 0).reshape(D, Tsteps * bl)   # [D, (t, b)]
    mtr = m.transpose(2, 1, 0).reshape(D, Tsteps * bl)
    return dict(xt=xtr.astype(BF), mt=mtr.astype(np.uint8))


_NC_CACHE = {}


def kernel(x, k0, r0, b0, k1, r1, b1, W, bias):
    x = np.asarray(x, np.float32)
    if "nc" not in _NC_CACHE:
        _NC_CACHE["nc"] = build()
    nc = _NC_CACHE["nc"]
    shared = prep_shared(k0, r0, b0, k1, r1, b1, W, bias)
    in_maps = []
    for c in range(NCORES):
        m = dict(shared)
        m.update(prep_core(x[c * BL:(c + 1) * BL]))
        in_maps.append(m)
    res = run_bass_kernel_spmd(nc, in_maps, core_ids=list(range(NCORES)))
    pred = np.concatenate([res.results[c]["pred_out"] for c in range(NCORES)], axis=0)
    lc = np.concatenate([res.results[c]["lc_out"] for c in range(NCORES)], axis=0)
    return pred.astype(np.float32), lc.astype(np.float32)
